# revision 42
# baseline (speedup 1.0000x reference)
"""Trainium2 Bass kernel for nn_MultiHeadAttention (N=8, S=1024, E=1024, H=16).

Strategy: pure data-parallel over the batch dim N=8 -> one batch element per
NeuronCore, no collectives. Per core the whole MHA runs out of SBUF:

  v   = xv @ Wv_aug.T + bv_aug   (S-major [S, H*(D+1)] with a ones column
                                  appended per head -> o-matmul also yields
                                  the softmax denominator for free)
  q.T = Wq @ xq.T + bq      (E-major "transposed" layout [E, S])
  k.T = Wk @ xk.T + bk
  per head h (software-pipelined at s_k-tile granularity):
    scoresT[s_k, s_q] tile = k_h.T-slice x q_h.T   (PSUM)
    attnT = exp(scoresT/sqrt(E))  on ScalarE (the only exp engine; paces
                                   the attention phase at ~1us/tile)
    o_unnorm.T[d, s_q] (+ denom row) += v_aug_h^T @ attnT  (PSUM accum)
  normalization is deferred/batched: denominator rows collect in SBUF, a
  fast approx reciprocal runs per batch of heads, the reciprocal rows are
  broadcast across partitions with a tiny K=2 fp32 matmul (selector
  constant), and oT is scaled in place -- all off the PE critical path.
  out = o @ Wo.T + bo       (natural [S, E] layout, DMA to DRAM)

All big matmul operands are bf16 (fp32 accumulation in PSUM); inputs are
pre-transposed and pre-cast on the host (layout/precision prep only).
"""

import math
from contextlib import ExitStack

import numpy as np

P = 128  # SBUF partitions
FDMAX = 512  # matmul moving-operand free-dim tile

_NC_CACHE = {}


def _emit(ctx, tc, io, S, E, H):
    from concourse import mybir

    nc = tc.nc
    D = E // H
    DA = D + 1
    HA = H * DA
    NTE = E // P  # partition tiles over e/f dims
    NTS = S // P  # partition tiles over s dim
    FD = min(FDMAX, S)
    NQ = S // FD  # free tiles over s
    NE = E // FD  # free tiles over e
    HPT = P // D  # heads per e-tile
    f32 = mybir.dt.float32
    bf16 = mybir.dt.bfloat16

    singles = ctx.enter_context(tc.tile_pool(name="singles", bufs=1))
    xpool = ctx.enter_context(tc.tile_pool(name="xpool", bufs=2))
    wpool = ctx.enter_context(tc.tile_pool(name="wpool", bufs=2))
    atp = ctx.enter_context(tc.tile_pool(name="atp", bufs=6))
    outp = ctx.enter_context(tc.tile_pool(name="outp", bufs=2))
    mini = ctx.enter_context(tc.tile_pool(name="mini", bufs=2))
    psA = ctx.enter_context(tc.tile_pool(name="psA", bufs=2, space="PSUM"))
    psO = ctx.enter_context(tc.tile_pool(name="psO", bufs=2, space="PSUM"))

    # persistent bf16 activations; layout [row % P, tile_idx * width + col]
    qT = singles.tile([P, NTE * S], bf16)  # q.T [e, s]
    kT = singles.tile([P, NTE * S], bf16)  # k.T [e, s]
    vA = singles.tile([P, NTS * HA], bf16)  # v_aug [s, HA]
    oT = singles.tile([P, NTE * S], bf16)  # o.T [e, s]

    # ---- input loads, in consumption order (DMA queues drain in order) ----
    xv_bf = xpool.tile([P, NTE * S], bf16, tag="x")
    wv_bf = wpool.tile([P, NTE * max(E, HA)], bf16, tag="w")
    for t in range(NTE):
        nc.sync.dma_start(
            out=wv_bf[:, t * HA : (t + 1) * HA], in_=io["wvTa"][t * P : (t + 1) * P, :]
        )
        nc.gpsimd.dma_start(
            out=xv_bf[:, t * S : (t + 1) * S], in_=io["xvT"][t * P : (t + 1) * P, :]
        )
    xq_bf = xpool.tile([P, NTE * S], bf16, tag="x")
    wq_bf = wpool.tile([P, NTE * max(E, HA)], bf16, tag="w")
    for t in range(NTE):
        nc.sync.dma_start(
            out=wq_bf[:, t * E : (t + 1) * E], in_=io["wqT"][t * P : (t + 1) * P, :]
        )
        nc.gpsimd.dma_start(
            out=xq_bf[:, t * S : (t + 1) * S], in_=io["xqT"][t * P : (t + 1) * P, :]
        )
    xk_bf = xpool.tile([P, NTE * S], bf16, tag="x")
    wk_bf = wpool.tile([P, NTE * max(E, HA)], bf16, tag="w")
    for t in range(NTE):
        nc.sync.dma_start(
            out=wk_bf[:, t * E : (t + 1) * E], in_=io["wkT"][t * P : (t + 1) * P, :]
        )
        nc.gpsimd.dma_start(
            out=xk_bf[:, t * S : (t + 1) * S], in_=io["xkT"][t * P : (t + 1) * P, :]
        )
    # wo is only needed at the tail -- keep it off the two main queues so
    # wk's last tiles arrive before the prefetched k-projection needs them
    wo_bf = wpool.tile([P, NTE * max(E, HA)], bf16, tag="w")
    for t in range(NTE):
        nc.scalar.dma_start(
            out=wo_bf[:, t * E : (t + 1) * E], in_=io["woT"][t * P : (t + 1) * P, :]
        )

    # biases + selector constant (scalar queue; small)
    bq_sb = singles.tile([P, NTE], f32)
    bk_sb = singles.tile([P, NTE], f32)
    bq2 = io["bq"].rearrange("(a b) -> a b", b=1)
    bk2 = io["bk"].rearrange("(a b) -> a b", b=1)
    for et in range(NTE):
        nc.scalar.dma_start(
            out=bq_sb[:, et : et + 1], in_=bq2[et * P : (et + 1) * P, :]
        )
        nc.scalar.dma_start(
            out=bk_sb[:, et : et + 1], in_=bk2[et * P : (et + 1) * P, :]
        )
    bva_sb = singles.tile([P, HA], f32)
    nc.scalar.dma_start(
        out=bva_sb, in_=io["bva"].rearrange("(a b) -> a b", a=1).to_broadcast((P, HA))
    )
    bo_sb = singles.tile([P, E], f32)
    nc.scalar.dma_start(
        out=bo_sb, in_=io["bo"].rearrange("(a b) -> a b", a=1).to_broadcast((P, E))
    )
    sel_sb = singles.tile([HPT, P], bf16)
    nc.scalar.dma_start(out=sel_sb, in_=io["sel"])

    # warm up ScalarE's exp table (ACT_TABLE_LOAD ~2.7us) before attention
    warm = singles.tile([1, 8], f32)
    nc.vector.memset(warm, 0.0)
    nc.scalar.activation(
        out=warm, in_=warm, func=mybir.ActivationFunctionType.Exp, scale=1.0
    )

    # ---- v projection: v_aug[s, c] = sum_f xv[f, s] * wv_aug[f, c] + bva ----
    # st-tiles in pairs with kt OUTER so the first matmuls consume xv/wv
    # tiles as the DMAs stream in (instead of waiting for the full 8MB)
    main_w = (HA // FD) * FD
    tail_w = HA - main_w
    SG = 2 if NTS % 2 == 0 else 1
    for stp in range(NTS // SG):
        sts = [stp * SG + i for i in range(SG)]
        mains = [
            psA.tile([P, max(S, main_w)], f32, tag="psA", name=f"ps_vm_{s}")
            for s in sts
        ]
        tails = [
            psO.tile([P, max(S, main_w)], f32, tag="psO", name=f"ps_vt_{s}")
            if tail_w
            else None
            for s in sts
        ]
        for kt in range(NTE):
            for ps_main, ps_tail, st_i in zip(mains, tails, sts):
                lhsT = xv_bf[:, kt * S + st_i * P : kt * S + st_i * P + P]
                for j in range(main_w // FD):
                    nc.tensor.matmul(
                        ps_main[:, j * FD : (j + 1) * FD],
                        lhsT,
                        wv_bf[:, kt * HA + j * FD : kt * HA + (j + 1) * FD],
                        start=(kt == 0),
                        stop=(kt == NTE - 1),
                    )
                if ps_tail is not None:
                    nc.tensor.matmul(
                        ps_tail[:, :tail_w],
                        lhsT,
                        wv_bf[:, kt * HA + main_w : kt * HA + HA],
                        start=(kt == 0),
                        stop=(kt == NTE - 1),
                    )
        for ps_main, ps_tail, st_i in zip(mains, tails, sts):
            nc.vector.tensor_add(
                out=vA[:, st_i * HA : st_i * HA + main_w],
                in0=ps_main[:, :main_w],
                in1=bva_sb[:, :main_w],
            )
            if ps_tail is not None:
                nc.vector.tensor_add(
                    out=vA[:, st_i * HA + main_w : (st_i + 1) * HA],
                    in0=ps_tail[:, :tail_w],
                    in1=bva_sb[:, main_w:HA],
                )

    # ---- q/k projection for ONE e-tile: dst[e,s] = sum_f w[f,e] x[f,s]+b ----
    def project_qk_et(dst, w_bf, x_bf, bias_sb, et, nm):
        ps = psO.tile([P, max(S, main_w)], f32, tag="psO", name=f"pqk_{nm}_{et}")
        for kt in range(NTE):
            lhsT = w_bf[:, kt * E + et * P : kt * E + (et + 1) * P]
            for j in range(NQ):
                nc.tensor.matmul(
                    ps[:, j * FD : (j + 1) * FD],
                    lhsT,
                    x_bf[:, kt * S + j * FD : kt * S + (j + 1) * FD],
                    start=(kt == 0),
                    stop=(kt == NTE - 1),
                )
        nc.vector.tensor_scalar_add(
            out=dst[:, et * S : (et + 1) * S],
            in0=ps[:, :S],
            scalar1=bias_sb[:, et : et + 1],
        )

    # ---- attention, kt-granular software pipeline per head ----
    inv_scale = 1.0 / math.sqrt(E)
    nbatch = int(__import__("os").environ.get("MHA_NBATCH", "8"))
    if H % nbatch != 0:
        nbatch = 1
    hb = H // nbatch
    # den tiles rotate through the mini pool: only batches b and b-1 are
    # alive at once (delay-1 normalization), so bufs=2 suffices
    den_batches = {}
    den_bfs = {}

    def den_tiles(b):
        if b not in den_batches:
            den_batches[b] = mini.tile(
                [hb, S], f32, tag="den_f32", name=f"den_batch{b}"
            )
            den_bfs[b] = mini.tile([hb, S], bf16, tag="den_bf", name=f"den_bf{b}")
        return den_batches[b], den_bfs[b]

    def normalize_et(et):
        # rb[p, s] = 1/den[head(p), s] replicated via K=HPT fp32 matmul
        den2 = mini.tile([HPT, S], bf16, tag="den2", name=f"den2_{et}")
        for i in range(HPT):
            hh = et * HPT + i
            nc.gpsimd.dma_start(
                out=den2[i : i + 1, :],
                in_=den_bfs[hh // hb][hh % hb : hh % hb + 1, :],
            )
        rb = psO.tile([P, max(S, main_w)], f32, tag="psO", name=f"rb_{et}")
        for j in range(NQ):
            nc.tensor.matmul(
                rb[:, j * FD : (j + 1) * FD],
                sel_sb,
                den2[:, j * FD : (j + 1) * FD],
                start=True,
                stop=True,
            )
        nc.vector.tensor_mul(
            out=oT[:, et * S : (et + 1) * S],
            in0=oT[:, et * S : (et + 1) * S],
            in1=rb[:, :S],
        )

    # prefetch first e-tile's q/k so attention can start immediately;
    # later e-tiles are projected inside the attention stream (keeps PE
    # 100% busy through the ScalarE-paced exp phase -> HAM stays warm)
    project_qk_et(qT, wq_bf, xq_bf, bq_sb, 0, "q")
    project_qk_et(kT, wk_bf, xk_bf, bk_sb, 0, "k")
    if NTE > 1:
        project_qk_et(qT, wq_bf, xq_bf, bq_sb, 1, "q")
        project_qk_et(kT, wk_bf, xk_bf, bk_sb, 1, "k")

    for h in range(H):
        eh = (h * D) // P
        ph = (h * D) % P
        pso = psO.tile([P, max(S, main_w)], f32, tag="psO", name=f"pso_{h}")
        ats = []
        # scores/exp for kt, with o-matmuls trailing one kt behind
        for kt in range(NTS):
            ps = psA.tile([P, max(S, main_w)], f32, tag="psA", name=f"sc_{h}_{kt}")
            lhsT = kT[ph : ph + D, eh * S + kt * P : eh * S + (kt + 1) * P]
            for j in range(NQ):
                nc.tensor.matmul(
                    ps[:, j * FD : (j + 1) * FD],
                    lhsT,
                    qT[ph : ph + D, eh * S + j * FD : eh * S + (j + 1) * FD],
                    start=True,
                    stop=True,
                )
            at = atp.tile([P, S], bf16, tag="at", name=f"at_{h}_{kt}")
            nc.scalar.activation(
                out=at,
                in_=ps[:, :S],
                func=mybir.ActivationFunctionType.Exp,
                scale=inv_scale,
            )
            ats.append(at)

            def o_mms(okt):
                lhsTo = vA[:, okt * HA + h * DA : okt * HA + (h + 1) * DA]
                for j in range(NQ):
                    nc.tensor.matmul(
                        pso[:DA, j * FD : (j + 1) * FD],
                        lhsTo,
                        ats[okt][:, j * FD : (j + 1) * FD],
                        start=(okt == 0),
                        stop=(okt == NTS - 1),
                    )

            if kt >= 2:
                o_mms(kt - 2)
        o_mms(NTS - 2)
        o_mms(NTS - 1)
        # evacuate unnormalized o (bf16) + denominator row
        nc.vector.tensor_copy(
            out=oT[ph : ph + D, eh * S : (eh + 1) * S], in_=pso[:D, :S]
        )
        den_tmp = mini.tile([1, S], f32, tag="den_tmp")
        nc.vector.tensor_copy(out=den_tmp, in_=pso[D:DA, :S])
        den_f32_b, _ = den_tiles(h // hb)
        nc.gpsimd.dma_start(
            out=den_f32_b[h % hb : h % hb + 1, :], in_=den_tmp
        )
        if h % hb == hb - 1:
            b = h // hb
            nc.vector.reciprocal_approx_fast(
                out=den_batches[b], in_=den_batches[b]
            )
            nc.vector.tensor_copy(out=den_bfs[b], in_=den_batches[b])
            # normalize with ONE BATCH of delay: batch b-1's reciprocal chain
            # (DVE+DMA) has had a full batch of PE work to complete, so the
            # rb matmuls never stall the in-order PE queue.
            if b > 0:
                for et in range(((b - 1) * hb) // HPT, (b * hb) // HPT):
                    normalize_et(et)
            if h == H - 1:
                for et in range((b * hb) // HPT, (h + 1) // HPT):
                    normalize_et(et)
        # between the two heads of a pair, project the e-tile TWO ahead
        if h % HPT == 0 and eh + 2 < NTE:
            project_qk_et(qT, wq_bf, xq_bf, bq_sb, eh + 2, "q")
            project_qk_et(kT, wk_bf, xk_bf, bk_sb, eh + 2, "k")

    # ---- output projection: out[s, e] = sum_f oT[f, s] woT[f, e] + bo ----
    # The first two s-tiles split their kt loops: early kt matmuls (whose
    # oT e-tiles normalized long ago) run while the LAST norm batch's
    # reciprocal chain completes, hiding it.
    KSPLIT = max(NTE - HPT, 0)
    ost = {}

    def outproj_mms(st_i, kts, pool=None):
        if st_i not in ost:
            pl, tg = (pool, "psO") if pool is psO else (psA, "psA")
            ost[st_i] = (
                outp.tile([P, E], f32, tag="out", name=f"osb_{st_i}"),
                (pl or psA).tile(
                    [P, max(S, main_w)], f32, tag=tg, name=f"po_{st_i}"
                ),
            )
        osb, ps = ost[st_i]
        for kt in kts:
            lhsT = oT[:, kt * S + st_i * P : kt * S + st_i * P + P]
            for j in range(NE):
                nc.tensor.matmul(
                    ps[:, j * FD : (j + 1) * FD],
                    lhsT,
                    wo_bf[:, kt * E + j * FD : kt * E + (j + 1) * FD],
                    start=(kt == 0),
                    stop=(kt == NTE - 1),
                )
        if kts[-1] == NTE - 1:
            nc.vector.tensor_add(out=osb, in0=ps[:, :E], in1=bo_sb)
            nc.sync.dma_start(
                out=io["out"][st_i * P : (st_i + 1) * P, :], in_=osb
            )

    if NTS >= 4 and KSPLIT > 0:
        # four s-tiles' early-kt matmuls buffer the last norm batch's chain
        outproj_mms(0, list(range(KSPLIT)))
        outproj_mms(1, list(range(KSPLIT)))
        outproj_mms(2, list(range(KSPLIT)), pool=psO)
        outproj_mms(3, list(range(KSPLIT)), pool=psO)
        for s in range(4):
            outproj_mms(s, list(range(KSPLIT, NTE)))
        rest = range(4, NTS)
    elif NTS >= 2 and KSPLIT > 0:
        outproj_mms(0, list(range(KSPLIT)))
        outproj_mms(1, list(range(KSPLIT)))
        outproj_mms(0, list(range(KSPLIT, NTE)))
        outproj_mms(1, list(range(KSPLIT, NTE)))
        rest = range(2, NTS)
    else:
        rest = range(NTS)
    for st_i in rest:
        outproj_mms(st_i, list(range(NTE)))


def build_nc(S=1024, E=1024, H=16):
    key = (S, E, H)
    if key in _NC_CACHE:
        return _NC_CACHE[key]
    import concourse.tile as tile
    from concourse import bacc, mybir

    D = E // H
    HA = H * (D + 1)
    HPT = P // D
    f32 = mybir.dt.float32
    bf16 = mybir.dt.bfloat16
    nc = bacc.Bacc("TRN2", target_bir_lowering=False, debug=False)
    io = {}
    for name, shape, dt in [
        ("xqT", [E, S], bf16),
        ("xkT", [E, S], bf16),
        ("xvT", [E, S], bf16),
        ("wqT", [E, E], bf16),
        ("wkT", [E, E], bf16),
        ("wvTa", [E, HA], bf16),
        ("woT", [E, E], bf16),
        ("bq", [E], f32),
        ("bk", [E], f32),
        ("bva", [HA], f32),
        ("bo", [E], f32),
        ("sel", [HPT, P], bf16),
    ]:
        io[name] = nc.dram_tensor(name, shape, dt, kind="ExternalInput").ap()
    io["out"] = nc.dram_tensor("out", [S, E], f32, kind="ExternalOutput").ap()

    with tile.TileContext(nc) as tc:
        with ExitStack() as ctx:
            _emit(ctx, tc, io, S, E, H)
    nc.compile()
    _NC_CACHE[key] = nc
    return nc


def make_in_maps(queries, keys, values, Wq, bq, Wk, bk, Wv, bv, Wo, bo, H=16):
    """Host-side layout prep: transposes, bf16 casts, v augmentation."""
    import ml_dtypes

    N, S, E = queries.shape
    D = E // H
    DA = D + 1
    HA = H * DA
    HPT = P // D
    f32 = np.float32
    bf16 = ml_dtypes.bfloat16

    wqT = np.ascontiguousarray(np.asarray(Wq, f32).T.astype(bf16))
    wkT = np.ascontiguousarray(np.asarray(Wk, f32).T.astype(bf16))
    woT = np.ascontiguousarray(np.asarray(Wo, f32).T.astype(bf16))
    wvT = np.asarray(Wv, f32).T.astype(bf16)  # [f, e]
    wvTa = np.zeros((E, HA), bf16)
    bva = np.zeros((HA,), f32)
    bv = np.asarray(bv, f32)
    for h in range(H):
        wvTa[:, h * DA : h * DA + D] = wvT[:, h * D : (h + 1) * D]
        bva[h * DA : h * DA + D] = bv[h * D : (h + 1) * D]
        bva[h * DA + D] = 1.0  # ones column -> softmax denominator
    sel = np.zeros((HPT, P), bf16)
    for i in range(HPT):
        sel[i, i * D : (i + 1) * D] = 1.0
    shared = {
        "wqT": wqT,
        "wkT": wkT,
        "wvTa": wvTa,
        "woT": woT,
        "bq": np.ascontiguousarray(np.asarray(bq, f32)),
        "bk": np.ascontiguousarray(np.asarray(bk, f32)),
        "bva": bva,
        "bo": np.ascontiguousarray(np.asarray(bo, f32)),
        "sel": sel,
    }
    q = np.asarray(queries, f32)
    k = np.asarray(keys, f32)
    v = np.asarray(values, f32)
    in_maps = []
    for b in range(N):
        m = dict(shared)
        m["xqT"] = np.ascontiguousarray(q[b].T.astype(bf16))
        m["xkT"] = np.ascontiguousarray(k[b].T.astype(bf16))
        m["xvT"] = np.ascontiguousarray(v[b].T.astype(bf16))
        in_maps.append(m)
    return in_maps


def run(queries, keys, values, Wq, bq, Wk, bk, Wv, bv, Wo, bo, **spmd_kwargs):
    from concourse.bass_utils import run_bass_kernel_spmd

    queries = np.asarray(queries, np.float32)
    N, S, E = queries.shape
    H = 16
    nc = build_nc(S=S, E=E, H=H)
    in_maps = make_in_maps(queries, keys, values, Wq, bq, Wk, bk, Wv, bv, Wo, bo, H=H)
    res = run_bass_kernel_spmd(nc, in_maps, core_ids=list(range(N)), **spmd_kwargs)
    out = np.stack([res.results[b]["out"] for b in range(N)])
    return out.astype(np.float32), res


def kernel(queries, keys, values, Wq, bq, Wk, bk, Wv, bv, Wo, bo):
    out, _ = run(queries, keys, values, Wq, bq, Wk, bk, Wv, bv, Wo, bo)
    return out


# revision 43
# speedup vs baseline: 1.0622x; 1.0622x over previous
"""Trainium2 Bass kernel for nn_MultiHeadAttention (N=8, S=1024, E=1024, H=16).

Strategy: pure data-parallel over the batch dim N=8 -> one batch element per
NeuronCore, no collectives. Per core the whole MHA runs out of SBUF:

  v   = xv @ Wv_aug.T + bv_aug   (S-major [S, H*(D+1)] with a ones column
                                  appended per head -> o-matmul also yields
                                  the softmax denominator for free)
  q.T = Wq @ xq.T + bq      (E-major "transposed" layout [E, S])
  k.T = Wk @ xk.T + bk
  per head h (software-pipelined at s_k-tile granularity):
    scoresT[s_k, s_q] tile = k_h.T-slice x q_h.T   (PSUM)
    attnT = exp(scoresT/sqrt(E))  on ScalarE (the only exp engine; paces
                                   the attention phase at ~1us/tile)
    o_unnorm.T[d, s_q] (+ denom row) += v_aug_h^T @ attnT  (PSUM accum)
  normalization is deferred/batched: denominator rows collect in SBUF, a
  fast approx reciprocal runs per batch of heads, the reciprocal rows are
  broadcast across partitions with a tiny K=2 fp32 matmul (selector
  constant), and oT is scaled in place -- all off the PE critical path.
  out = o @ Wo.T + bo       (natural [S, E] layout, DMA to DRAM)

All big matmul operands are bf16 (fp32 accumulation in PSUM); inputs are
pre-transposed and pre-cast on the host (layout/precision prep only).
"""

import math
from contextlib import ExitStack

import numpy as np

P = 128  # SBUF partitions
FDMAX = 512  # matmul moving-operand free-dim tile

_NC_CACHE = {}


def _emit(ctx, tc, io, S, E, H):
    from concourse import mybir

    nc = tc.nc
    D = E // H
    DA = D + 1
    HA = H * DA
    NTE = E // P  # partition tiles over e/f dims
    NTS = S // P  # partition tiles over s dim
    FD = min(FDMAX, S)
    NQ = S // FD  # free tiles over s
    NE = E // FD  # free tiles over e
    HPT = P // D  # heads per e-tile
    f32 = mybir.dt.float32
    bf16 = mybir.dt.bfloat16

    singles = ctx.enter_context(tc.tile_pool(name="singles", bufs=1))
    xpool = ctx.enter_context(tc.tile_pool(name="xpool", bufs=2))
    wpool = ctx.enter_context(tc.tile_pool(name="wpool", bufs=2))
    atp = ctx.enter_context(tc.tile_pool(name="atp", bufs=6))
    outp = ctx.enter_context(tc.tile_pool(name="outp", bufs=2))
    mini = ctx.enter_context(tc.tile_pool(name="mini", bufs=2))
    psA = ctx.enter_context(tc.tile_pool(name="psA", bufs=2, space="PSUM"))
    psO = ctx.enter_context(tc.tile_pool(name="psO", bufs=2, space="PSUM"))

    # persistent bf16 activations; layout [row % P, tile_idx * width + col]
    qT = singles.tile([P, NTE * S], bf16)  # q.T [e, s]
    kT = singles.tile([P, NTE * S], bf16)  # k.T [e, s]
    vA = singles.tile([P, NTS * HA], bf16)  # v_aug [s, HA]
    oT = singles.tile([P, NTE * S], bf16)  # o.T [e, s]

    # ---- input loads, in consumption order (DMA queues drain in order) ----
    xv_bf = xpool.tile([P, NTE * S], bf16, tag="x")
    wv_bf = wpool.tile([P, NTE * max(E, HA)], bf16, tag="w")
    for t in range(NTE):
        nc.sync.dma_start(
            out=wv_bf[:, t * HA : (t + 1) * HA], in_=io["wvTa"][t * P : (t + 1) * P, :]
        )
        nc.gpsimd.dma_start(
            out=xv_bf[:, t * S : (t + 1) * S], in_=io["xvT"][t * P : (t + 1) * P, :]
        )
    xq_bf = xpool.tile([P, NTE * S], bf16, tag="x")
    wq_bf = wpool.tile([P, NTE * max(E, HA)], bf16, tag="w")
    for t in range(NTE):
        nc.sync.dma_start(
            out=wq_bf[:, t * E : (t + 1) * E], in_=io["wqT"][t * P : (t + 1) * P, :]
        )
        nc.gpsimd.dma_start(
            out=xq_bf[:, t * S : (t + 1) * S], in_=io["xqT"][t * P : (t + 1) * P, :]
        )
    xk_bf = xpool.tile([P, NTE * S], bf16, tag="x")
    wk_bf = wpool.tile([P, NTE * max(E, HA)], bf16, tag="w")
    for t in range(NTE):
        nc.sync.dma_start(
            out=wk_bf[:, t * E : (t + 1) * E], in_=io["wkT"][t * P : (t + 1) * P, :]
        )
        nc.gpsimd.dma_start(
            out=xk_bf[:, t * S : (t + 1) * S], in_=io["xkT"][t * P : (t + 1) * P, :]
        )
    # biases + selector constant (scalar queue; small)
    bq_sb = singles.tile([P, NTE], f32)
    bk_sb = singles.tile([P, NTE], f32)
    bq2 = io["bq"].rearrange("(a b) -> a b", b=1)
    bk2 = io["bk"].rearrange("(a b) -> a b", b=1)
    for et in range(NTE):
        nc.scalar.dma_start(
            out=bq_sb[:, et : et + 1], in_=bq2[et * P : (et + 1) * P, :]
        )
        nc.scalar.dma_start(
            out=bk_sb[:, et : et + 1], in_=bk2[et * P : (et + 1) * P, :]
        )
    bva_sb = singles.tile([P, HA], f32)
    nc.scalar.dma_start(
        out=bva_sb, in_=io["bva"].rearrange("(a b) -> a b", a=1).to_broadcast((P, HA))
    )
    bo_sb = singles.tile([P, E], f32)
    nc.scalar.dma_start(
        out=bo_sb, in_=io["bo"].rearrange("(a b) -> a b", a=1).to_broadcast((P, E))
    )
    sel_sb = singles.tile([HPT, P], bf16)
    nc.scalar.dma_start(out=sel_sb, in_=io["sel"])

    # warm up ScalarE's exp table (ACT_TABLE_LOAD ~2.7us) before attention
    warm = singles.tile([1, 8], f32)
    nc.vector.memset(warm, 0.0)
    nc.scalar.activation(
        out=warm, in_=warm, func=mybir.ActivationFunctionType.Exp, scale=1.0
    )

    # wo is only needed at the tail -- keep it off the two main queues so
    # wk's last tiles arrive before the prefetched k-projection needs them
    wo_bf = wpool.tile([P, NTE * max(E, HA)], bf16, tag="w")
    for t in range(NTE):
        nc.scalar.dma_start(
            out=wo_bf[:, t * E : (t + 1) * E], in_=io["woT"][t * P : (t + 1) * P, :]
        )


    # ---- v projection: v_aug[s, c] = sum_f xv[f, s] * wv_aug[f, c] + bva ----
    # st-tiles in pairs with kt OUTER so the first matmuls consume xv/wv
    # tiles as the DMAs stream in (instead of waiting for the full 8MB)
    main_w = (HA // FD) * FD
    tail_w = HA - main_w
    SG = 2 if NTS % 2 == 0 else 1
    for stp in range(NTS // SG):
        sts = [stp * SG + i for i in range(SG)]
        mains = [
            psA.tile([P, max(S, main_w)], f32, tag="psA", name=f"ps_vm_{s}")
            for s in sts
        ]
        tails = [
            psO.tile([P, max(S, main_w)], f32, tag="psO", name=f"ps_vt_{s}")
            if tail_w
            else None
            for s in sts
        ]
        for kt in range(NTE):
            for ps_main, ps_tail, st_i in zip(mains, tails, sts):
                lhsT = xv_bf[:, kt * S + st_i * P : kt * S + st_i * P + P]
                for j in range(main_w // FD):
                    nc.tensor.matmul(
                        ps_main[:, j * FD : (j + 1) * FD],
                        lhsT,
                        wv_bf[:, kt * HA + j * FD : kt * HA + (j + 1) * FD],
                        start=(kt == 0),
                        stop=(kt == NTE - 1),
                    )
                if ps_tail is not None:
                    nc.tensor.matmul(
                        ps_tail[:, :tail_w],
                        lhsT,
                        wv_bf[:, kt * HA + main_w : kt * HA + HA],
                        start=(kt == 0),
                        stop=(kt == NTE - 1),
                    )
        for ps_main, ps_tail, st_i in zip(mains, tails, sts):
            nc.vector.tensor_add(
                out=vA[:, st_i * HA : st_i * HA + main_w],
                in0=ps_main[:, :main_w],
                in1=bva_sb[:, :main_w],
            )
            if ps_tail is not None:
                nc.vector.tensor_add(
                    out=vA[:, st_i * HA + main_w : (st_i + 1) * HA],
                    in0=ps_tail[:, :tail_w],
                    in1=bva_sb[:, main_w:HA],
                )

    # ---- q/k projection for ONE e-tile: dst[e,s] = sum_f w[f,e] x[f,s]+b ----
    def project_qk_et(dst, w_bf, x_bf, bias_sb, et, nm):
        ps = psO.tile([P, max(S, main_w)], f32, tag="psO", name=f"pqk_{nm}_{et}")
        for kt in range(NTE):
            lhsT = w_bf[:, kt * E + et * P : kt * E + (et + 1) * P]
            for j in range(NQ):
                nc.tensor.matmul(
                    ps[:, j * FD : (j + 1) * FD],
                    lhsT,
                    x_bf[:, kt * S + j * FD : kt * S + (j + 1) * FD],
                    start=(kt == 0),
                    stop=(kt == NTE - 1),
                )
        nc.vector.tensor_scalar_add(
            out=dst[:, et * S : (et + 1) * S],
            in0=ps[:, :S],
            scalar1=bias_sb[:, et : et + 1],
        )

    # ---- attention, kt-granular software pipeline per head ----
    inv_scale = 1.0 / math.sqrt(E)
    nbatch = int(__import__("os").environ.get("MHA_NBATCH", "8"))
    if H % nbatch != 0:
        nbatch = 1
    hb = H // nbatch
    # den tiles rotate through the mini pool: only batches b and b-1 are
    # alive at once (delay-1 normalization), so bufs=2 suffices
    den_batches = {}
    den_bfs = {}

    def den_tiles(b):
        if b not in den_batches:
            den_batches[b] = mini.tile(
                [hb, S], f32, tag="den_f32", name=f"den_batch{b}"
            )
            den_bfs[b] = mini.tile([hb, S], bf16, tag="den_bf", name=f"den_bf{b}")
        return den_batches[b], den_bfs[b]

    def normalize_et(et):
        # rb[p, s] = 1/den[head(p), s] replicated via K=HPT fp32 matmul
        den2 = mini.tile([HPT, S], bf16, tag="den2", name=f"den2_{et}")
        for i in range(HPT):
            hh = et * HPT + i
            nc.gpsimd.dma_start(
                out=den2[i : i + 1, :],
                in_=den_bfs[hh // hb][hh % hb : hh % hb + 1, :],
            )
        rb = psO.tile([P, max(S, main_w)], f32, tag="psO", name=f"rb_{et}")
        for j in range(NQ):
            nc.tensor.matmul(
                rb[:, j * FD : (j + 1) * FD],
                sel_sb,
                den2[:, j * FD : (j + 1) * FD],
                start=True,
                stop=True,
            )
        nc.vector.tensor_mul(
            out=oT[:, et * S : (et + 1) * S],
            in0=oT[:, et * S : (et + 1) * S],
            in1=rb[:, :S],
        )

    # prefetch first e-tile's q/k so attention can start immediately;
    # later e-tiles are projected inside the attention stream (keeps PE
    # 100% busy through the ScalarE-paced exp phase -> HAM stays warm)
    project_qk_et(qT, wq_bf, xq_bf, bq_sb, 0, "q")
    project_qk_et(kT, wk_bf, xk_bf, bk_sb, 0, "k")
    if NTE > 1:
        project_qk_et(qT, wq_bf, xq_bf, bq_sb, 1, "q")
        project_qk_et(kT, wk_bf, xk_bf, bk_sb, 1, "k")

    for h in range(H):
        eh = (h * D) // P
        ph = (h * D) % P
        pso = psO.tile([P, max(S, main_w)], f32, tag="psO", name=f"pso_{h}")
        ats = []
        # scores/exp for kt, with o-matmuls trailing one kt behind
        for kt in range(NTS):
            ps = psA.tile([P, max(S, main_w)], f32, tag="psA", name=f"sc_{h}_{kt}")
            lhsT = kT[ph : ph + D, eh * S + kt * P : eh * S + (kt + 1) * P]
            for j in range(NQ):
                nc.tensor.matmul(
                    ps[:, j * FD : (j + 1) * FD],
                    lhsT,
                    qT[ph : ph + D, eh * S + j * FD : eh * S + (j + 1) * FD],
                    start=True,
                    stop=True,
                )
            at = atp.tile([P, S], bf16, tag="at", name=f"at_{h}_{kt}")
            nc.scalar.activation(
                out=at,
                in_=ps[:, :S],
                func=mybir.ActivationFunctionType.Exp,
                scale=inv_scale,
            )
            ats.append(at)

            def o_mms(okt):
                lhsTo = vA[:, okt * HA + h * DA : okt * HA + (h + 1) * DA]
                for j in range(NQ):
                    nc.tensor.matmul(
                        pso[:DA, j * FD : (j + 1) * FD],
                        lhsTo,
                        ats[okt][:, j * FD : (j + 1) * FD],
                        start=(okt == 0),
                        stop=(okt == NTS - 1),
                    )

            if kt >= 2:
                o_mms(kt - 2)
        o_mms(NTS - 2)
        o_mms(NTS - 1)
        # evacuate unnormalized o (bf16) + denominator row
        nc.vector.tensor_copy(
            out=oT[ph : ph + D, eh * S : (eh + 1) * S], in_=pso[:D, :S]
        )
        den_tmp = mini.tile([1, S], f32, tag="den_tmp")
        nc.vector.tensor_copy(out=den_tmp, in_=pso[D:DA, :S])
        den_f32_b, _ = den_tiles(h // hb)
        nc.gpsimd.dma_start(
            out=den_f32_b[h % hb : h % hb + 1, :], in_=den_tmp
        )
        if h % hb == hb - 1:
            b = h // hb
            nc.vector.reciprocal_approx_fast(
                out=den_batches[b], in_=den_batches[b]
            )
            nc.vector.tensor_copy(out=den_bfs[b], in_=den_batches[b])
            # normalize with ONE BATCH of delay: batch b-1's reciprocal chain
            # (DVE+DMA) has had a full batch of PE work to complete, so the
            # rb matmuls never stall the in-order PE queue.
            if b > 0:
                for et in range(((b - 1) * hb) // HPT, (b * hb) // HPT):
                    normalize_et(et)
            if h == H - 1:
                for et in range((b * hb) // HPT, (h + 1) // HPT):
                    normalize_et(et)
        # between the two heads of a pair, project the e-tile TWO ahead
        if h % HPT == 0 and eh + 2 < NTE:
            project_qk_et(qT, wq_bf, xq_bf, bq_sb, eh + 2, "q")
            project_qk_et(kT, wk_bf, xk_bf, bk_sb, eh + 2, "k")

    # ---- output projection: out[s, e] = sum_f oT[f, s] woT[f, e] + bo ----
    # The first two s-tiles split their kt loops: early kt matmuls (whose
    # oT e-tiles normalized long ago) run while the LAST norm batch's
    # reciprocal chain completes, hiding it.
    KSPLIT = max(NTE - HPT, 0)
    ost = {}

    def outproj_mms(st_i, kts, pool=None):
        if st_i not in ost:
            pl, tg = (pool, "psO") if pool is psO else (psA, "psA")
            ost[st_i] = (
                outp.tile([P, E], f32, tag="out", name=f"osb_{st_i}"),
                (pl or psA).tile(
                    [P, max(S, main_w)], f32, tag=tg, name=f"po_{st_i}"
                ),
            )
        osb, ps = ost[st_i]
        for kt in kts:
            lhsT = oT[:, kt * S + st_i * P : kt * S + st_i * P + P]
            for j in range(NE):
                nc.tensor.matmul(
                    ps[:, j * FD : (j + 1) * FD],
                    lhsT,
                    wo_bf[:, kt * E + j * FD : kt * E + (j + 1) * FD],
                    start=(kt == 0),
                    stop=(kt == NTE - 1),
                )
        if kts[-1] == NTE - 1:
            nc.vector.tensor_add(out=osb, in0=ps[:, :E], in1=bo_sb)
            nc.sync.dma_start(
                out=io["out"][st_i * P : (st_i + 1) * P, :], in_=osb
            )

    if NTS >= 4 and KSPLIT > 0:
        # four s-tiles' early-kt matmuls buffer the last norm batch's chain
        outproj_mms(0, list(range(KSPLIT)))
        outproj_mms(1, list(range(KSPLIT)))
        outproj_mms(2, list(range(KSPLIT)), pool=psO)
        outproj_mms(3, list(range(KSPLIT)), pool=psO)
        for s in range(4):
            outproj_mms(s, list(range(KSPLIT, NTE)))
        rest = range(4, NTS)
    elif NTS >= 2 and KSPLIT > 0:
        outproj_mms(0, list(range(KSPLIT)))
        outproj_mms(1, list(range(KSPLIT)))
        outproj_mms(0, list(range(KSPLIT, NTE)))
        outproj_mms(1, list(range(KSPLIT, NTE)))
        rest = range(2, NTS)
    else:
        rest = range(NTS)
    for st_i in rest:
        outproj_mms(st_i, list(range(NTE)))


def build_nc(S=1024, E=1024, H=16):
    key = (S, E, H)
    if key in _NC_CACHE:
        return _NC_CACHE[key]
    import concourse.tile as tile
    from concourse import bacc, mybir

    D = E // H
    HA = H * (D + 1)
    HPT = P // D
    f32 = mybir.dt.float32
    bf16 = mybir.dt.bfloat16
    nc = bacc.Bacc("TRN2", target_bir_lowering=False, debug=False)
    io = {}
    for name, shape, dt in [
        ("xqT", [E, S], bf16),
        ("xkT", [E, S], bf16),
        ("xvT", [E, S], bf16),
        ("wqT", [E, E], bf16),
        ("wkT", [E, E], bf16),
        ("wvTa", [E, HA], bf16),
        ("woT", [E, E], bf16),
        ("bq", [E], f32),
        ("bk", [E], f32),
        ("bva", [HA], f32),
        ("bo", [E], f32),
        ("sel", [HPT, P], bf16),
    ]:
        io[name] = nc.dram_tensor(name, shape, dt, kind="ExternalInput").ap()
    io["out"] = nc.dram_tensor("out", [S, E], f32, kind="ExternalOutput").ap()

    with tile.TileContext(nc) as tc:
        with ExitStack() as ctx:
            _emit(ctx, tc, io, S, E, H)
    nc.compile()
    _NC_CACHE[key] = nc
    return nc


def make_in_maps(queries, keys, values, Wq, bq, Wk, bk, Wv, bv, Wo, bo, H=16):
    """Host-side layout prep: transposes, bf16 casts, v augmentation."""
    import ml_dtypes

    N, S, E = queries.shape
    D = E // H
    DA = D + 1
    HA = H * DA
    HPT = P // D
    f32 = np.float32
    bf16 = ml_dtypes.bfloat16

    wqT = np.ascontiguousarray(np.asarray(Wq, f32).T.astype(bf16))
    wkT = np.ascontiguousarray(np.asarray(Wk, f32).T.astype(bf16))
    woT = np.ascontiguousarray(np.asarray(Wo, f32).T.astype(bf16))
    wvT = np.asarray(Wv, f32).T.astype(bf16)  # [f, e]
    wvTa = np.zeros((E, HA), bf16)
    bva = np.zeros((HA,), f32)
    bv = np.asarray(bv, f32)
    for h in range(H):
        wvTa[:, h * DA : h * DA + D] = wvT[:, h * D : (h + 1) * D]
        bva[h * DA : h * DA + D] = bv[h * D : (h + 1) * D]
        bva[h * DA + D] = 1.0  # ones column -> softmax denominator
    sel = np.zeros((HPT, P), bf16)
    for i in range(HPT):
        sel[i, i * D : (i + 1) * D] = 1.0
    shared = {
        "wqT": wqT,
        "wkT": wkT,
        "wvTa": wvTa,
        "woT": woT,
        "bq": np.ascontiguousarray(np.asarray(bq, f32)),
        "bk": np.ascontiguousarray(np.asarray(bk, f32)),
        "bva": bva,
        "bo": np.ascontiguousarray(np.asarray(bo, f32)),
        "sel": sel,
    }
    q = np.asarray(queries, f32)
    k = np.asarray(keys, f32)
    v = np.asarray(values, f32)
    in_maps = []
    for b in range(N):
        m = dict(shared)
        m["xqT"] = np.ascontiguousarray(q[b].T.astype(bf16))
        m["xkT"] = np.ascontiguousarray(k[b].T.astype(bf16))
        m["xvT"] = np.ascontiguousarray(v[b].T.astype(bf16))
        in_maps.append(m)
    return in_maps


def run(queries, keys, values, Wq, bq, Wk, bk, Wv, bv, Wo, bo, **spmd_kwargs):
    from concourse.bass_utils import run_bass_kernel_spmd

    queries = np.asarray(queries, np.float32)
    N, S, E = queries.shape
    H = 16
    nc = build_nc(S=S, E=E, H=H)
    in_maps = make_in_maps(queries, keys, values, Wq, bq, Wk, bk, Wv, bv, Wo, bo, H=H)
    res = run_bass_kernel_spmd(nc, in_maps, core_ids=list(range(N)), **spmd_kwargs)
    out = np.stack([res.results[b]["out"] for b in range(N)])
    return out.astype(np.float32), res


def kernel(queries, keys, values, Wq, bq, Wk, bk, Wv, bv, Wo, bo):
    out, _ = run(queries, keys, values, Wq, bq, Wk, bk, Wv, bv, Wo, bo)
    return out


# revision 45
# speedup vs baseline: 1.0950x; 1.0308x over previous
"""Trainium2 Bass kernel for nn_MultiHeadAttention (N=8, S=1024, E=1024, H=16).

Strategy: pure data-parallel over the batch dim N=8 -> one batch element per
NeuronCore, no collectives. Per core the whole MHA runs out of SBUF:

  v   = xv @ Wv_aug.T + bv_aug   (S-major [S, H*(D+1)] with a ones column
                                  appended per head -> o-matmul also yields
                                  the softmax denominator for free)
  q.T = Wq @ xq.T + bq      (E-major "transposed" layout [E, S])
  k.T = Wk @ xk.T + bk
  per head h (software-pipelined at s_k-tile granularity):
    scoresT[s_k, s_q] tile = k_h.T-slice x q_h.T   (PSUM)
    attnT = exp(scoresT/sqrt(E))  on ScalarE (the only exp engine; paces
                                   the attention phase at ~1us/tile)
    o_unnorm.T[d, s_q] (+ denom row) += v_aug_h^T @ attnT  (PSUM accum)
  normalization is deferred/batched: denominator rows collect in SBUF, a
  fast approx reciprocal runs per batch of heads, the reciprocal rows are
  broadcast across partitions with a tiny K=2 fp32 matmul (selector
  constant), and oT is scaled in place -- all off the PE critical path.
  out = o @ Wo.T + bo       (natural [S, E] layout, DMA to DRAM)

All big matmul operands are bf16 (fp32 accumulation in PSUM); inputs are
pre-transposed and pre-cast on the host (layout/precision prep only).
"""

import math
from contextlib import ExitStack

import numpy as np

P = 128  # SBUF partitions
FDMAX = 512  # matmul moving-operand free-dim tile

_NC_CACHE = {}


def _emit(ctx, tc, io, S, E, H):
    from concourse import mybir

    nc = tc.nc
    D = E // H
    DA = D + 1
    HA = H * DA
    NTE = E // P  # partition tiles over e/f dims
    NTS = S // P  # partition tiles over s dim
    FD = min(FDMAX, S)
    NQ = S // FD  # free tiles over s
    NE = E // FD  # free tiles over e
    HPT = P // D  # heads per e-tile
    f32 = mybir.dt.float32
    bf16 = mybir.dt.bfloat16

    singles = ctx.enter_context(tc.tile_pool(name="singles", bufs=1))
    xpool = ctx.enter_context(tc.tile_pool(name="xpool", bufs=2))
    wpool = ctx.enter_context(tc.tile_pool(name="wpool", bufs=2))
    atp = ctx.enter_context(tc.tile_pool(name="atp", bufs=6))
    outp = ctx.enter_context(tc.tile_pool(name="outp", bufs=2))
    mini = ctx.enter_context(tc.tile_pool(name="mini", bufs=2))
    psA = ctx.enter_context(tc.tile_pool(name="psA", bufs=2, space="PSUM"))
    psO = ctx.enter_context(tc.tile_pool(name="psO", bufs=2, space="PSUM"))

    # persistent bf16 activations; layout [row % P, tile_idx * width + col]
    qT = singles.tile([P, NTE * S], bf16)  # q.T [e, s]
    kT = singles.tile([P, NTE * S], bf16)  # k.T [e, s]
    vA = singles.tile([P, NTS * HA], bf16)  # v_aug [s, HA]
    oT = singles.tile([P, NTE * S], bf16)  # o.T [e, s]

    # ---- input loads, in consumption order (DMA queues drain in order) ----
    xv_bf = xpool.tile([P, NTE * S], bf16, tag="x")
    wv_bf = wpool.tile([P, NTE * max(E, HA)], bf16, tag="w")
    for t in range(NTE):
        ew = nc.sync if t % 2 == 0 else nc.gpsimd
        ex = nc.gpsimd if t % 2 == 0 else nc.sync
        ew.dma_start(
            out=wv_bf[:, t * HA : (t + 1) * HA], in_=io["wvTa"][t * P : (t + 1) * P, :]
        )
        ex.dma_start(
            out=xv_bf[:, t * S : (t + 1) * S], in_=io["xvT"][t * P : (t + 1) * P, :]
        )
    xq_bf = xpool.tile([P, NTE * S], bf16, tag="x")
    wq_bf = wpool.tile([P, NTE * max(E, HA)], bf16, tag="w")
    for t in range(NTE):
        ew = nc.sync if t % 2 == 0 else nc.gpsimd
        ex = nc.gpsimd if t % 2 == 0 else nc.sync
        ew.dma_start(
            out=wq_bf[:, t * E : (t + 1) * E], in_=io["wqT"][t * P : (t + 1) * P, :]
        )
        ex.dma_start(
            out=xq_bf[:, t * S : (t + 1) * S], in_=io["xqT"][t * P : (t + 1) * P, :]
        )
    xk_bf = xpool.tile([P, NTE * S], bf16, tag="x")
    wk_bf = wpool.tile([P, NTE * max(E, HA)], bf16, tag="w")
    for t in range(NTE):
        ew = nc.sync if t % 2 == 0 else nc.gpsimd
        ex = nc.gpsimd if t % 2 == 0 else nc.sync
        ew.dma_start(
            out=wk_bf[:, t * E : (t + 1) * E], in_=io["wkT"][t * P : (t + 1) * P, :]
        )
        ex.dma_start(
            out=xk_bf[:, t * S : (t + 1) * S], in_=io["xkT"][t * P : (t + 1) * P, :]
        )
    wo_bf = wpool.tile([P, NTE * max(E, HA)], bf16, tag="w")
    for t in range(NTE):
        eng = nc.sync if t % 2 == 0 else nc.gpsimd
        eng.dma_start(
            out=wo_bf[:, t * E : (t + 1) * E], in_=io["woT"][t * P : (t + 1) * P, :]
        )

    # biases + selector constant (scalar queue; small)
    bq_sb = singles.tile([P, NTE], f32)
    bk_sb = singles.tile([P, NTE], f32)
    bq2 = io["bq"].rearrange("(a b) -> a b", b=1)
    bk2 = io["bk"].rearrange("(a b) -> a b", b=1)
    for et in range(NTE):
        nc.scalar.dma_start(
            out=bq_sb[:, et : et + 1], in_=bq2[et * P : (et + 1) * P, :]
        )
        nc.scalar.dma_start(
            out=bk_sb[:, et : et + 1], in_=bk2[et * P : (et + 1) * P, :]
        )
    bva_sb = singles.tile([P, HA], f32)
    nc.scalar.dma_start(
        out=bva_sb, in_=io["bva"].rearrange("(a b) -> a b", a=1).to_broadcast((P, HA))
    )
    bo_sb = singles.tile([P, E], f32)
    nc.scalar.dma_start(
        out=bo_sb, in_=io["bo"].rearrange("(a b) -> a b", a=1).to_broadcast((P, E))
    )
    sel_sb = singles.tile([HPT, P], bf16)
    nc.scalar.dma_start(out=sel_sb, in_=io["sel"])

    # warm up ScalarE's exp table (ACT_TABLE_LOAD ~2.7us) before attention
    warm = singles.tile([1, 8], f32)
    nc.vector.memset(warm, 0.0)
    nc.scalar.activation(
        out=warm, in_=warm, func=mybir.ActivationFunctionType.Exp, scale=1.0
    )

    # ---- v projection: v_aug[s, c] = sum_f xv[f, s] * wv_aug[f, c] + bva ----
    # st-tiles in pairs with kt OUTER so the first matmuls consume xv/wv
    # tiles as the DMAs stream in (instead of waiting for the full 8MB)
    main_w = (HA // FD) * FD
    tail_w = HA - main_w
    SG = 2 if NTS % 2 == 0 else 1
    for stp in range(NTS // SG):
        sts = [stp * SG + i for i in range(SG)]
        mains = [
            psA.tile([P, max(S, main_w)], f32, tag="psA", name=f"ps_vm_{s}")
            for s in sts
        ]
        tails = [
            psO.tile([P, max(S, main_w)], f32, tag="psO", name=f"ps_vt_{s}")
            if tail_w
            else None
            for s in sts
        ]
        for kt in range(NTE):
            for ps_main, ps_tail, st_i in zip(mains, tails, sts):
                lhsT = xv_bf[:, kt * S + st_i * P : kt * S + st_i * P + P]
                for j in range(main_w // FD):
                    nc.tensor.matmul(
                        ps_main[:, j * FD : (j + 1) * FD],
                        lhsT,
                        wv_bf[:, kt * HA + j * FD : kt * HA + (j + 1) * FD],
                        start=(kt == 0),
                        stop=(kt == NTE - 1),
                    )
                if ps_tail is not None:
                    nc.tensor.matmul(
                        ps_tail[:, :tail_w],
                        lhsT,
                        wv_bf[:, kt * HA + main_w : kt * HA + HA],
                        start=(kt == 0),
                        stop=(kt == NTE - 1),
                    )
        for ps_main, ps_tail, st_i in zip(mains, tails, sts):
            nc.vector.tensor_add(
                out=vA[:, st_i * HA : st_i * HA + main_w],
                in0=ps_main[:, :main_w],
                in1=bva_sb[:, :main_w],
            )
            if ps_tail is not None:
                nc.vector.tensor_add(
                    out=vA[:, st_i * HA + main_w : (st_i + 1) * HA],
                    in0=ps_tail[:, :tail_w],
                    in1=bva_sb[:, main_w:HA],
                )

    # ---- q/k projection for ONE e-tile: dst[e,s] = sum_f w[f,e] x[f,s]+b ----
    def project_qk_et(dst, w_bf, x_bf, bias_sb, et, nm):
        ps = psO.tile([P, max(S, main_w)], f32, tag="psO", name=f"pqk_{nm}_{et}")
        for kt in range(NTE):
            lhsT = w_bf[:, kt * E + et * P : kt * E + (et + 1) * P]
            for j in range(NQ):
                nc.tensor.matmul(
                    ps[:, j * FD : (j + 1) * FD],
                    lhsT,
                    x_bf[:, kt * S + j * FD : kt * S + (j + 1) * FD],
                    start=(kt == 0),
                    stop=(kt == NTE - 1),
                )
        nc.vector.tensor_scalar_add(
            out=dst[:, et * S : (et + 1) * S],
            in0=ps[:, :S],
            scalar1=bias_sb[:, et : et + 1],
        )

    # ---- attention, kt-granular software pipeline per head ----
    inv_scale = 1.0 / math.sqrt(E)
    nbatch = int(__import__("os").environ.get("MHA_NBATCH", "8"))
    if H % nbatch != 0:
        nbatch = 1
    hb = H // nbatch
    # den tiles rotate through the mini pool: only batches b and b-1 are
    # alive at once (delay-1 normalization), so bufs=2 suffices
    den_batches = {}
    den_bfs = {}

    def den_tiles(b):
        if b not in den_batches:
            den_batches[b] = mini.tile(
                [hb, S], f32, tag="den_f32", name=f"den_batch{b}"
            )
            den_bfs[b] = mini.tile([hb, S], bf16, tag="den_bf", name=f"den_bf{b}")
        return den_batches[b], den_bfs[b]

    def normalize_et(et):
        # rb[p, s] = 1/den[head(p), s] replicated via K=HPT fp32 matmul
        den2 = mini.tile([HPT, S], bf16, tag="den2", name=f"den2_{et}")
        for i in range(HPT):
            hh = et * HPT + i
            nc.gpsimd.dma_start(
                out=den2[i : i + 1, :],
                in_=den_bfs[hh // hb][hh % hb : hh % hb + 1, :],
            )
        rb = psO.tile([P, max(S, main_w)], f32, tag="psO", name=f"rb_{et}")
        for j in range(NQ):
            nc.tensor.matmul(
                rb[:, j * FD : (j + 1) * FD],
                sel_sb,
                den2[:, j * FD : (j + 1) * FD],
                start=True,
                stop=True,
            )
        nc.vector.tensor_mul(
            out=oT[:, et * S : (et + 1) * S],
            in0=oT[:, et * S : (et + 1) * S],
            in1=rb[:, :S],
        )

    # prefetch first e-tile's q/k so attention can start immediately;
    # later e-tiles are projected inside the attention stream (keeps PE
    # 100% busy through the ScalarE-paced exp phase -> HAM stays warm)
    project_qk_et(qT, wq_bf, xq_bf, bq_sb, 0, "q")
    project_qk_et(kT, wk_bf, xk_bf, bk_sb, 0, "k")
    if NTE > 1:
        project_qk_et(qT, wq_bf, xq_bf, bq_sb, 1, "q")
        project_qk_et(kT, wk_bf, xk_bf, bk_sb, 1, "k")

    for h in range(H):
        eh = (h * D) // P
        ph = (h * D) % P
        pso = psO.tile([P, max(S, main_w)], f32, tag="psO", name=f"pso_{h}")
        ats = []
        # scores/exp for kt, with o-matmuls trailing one kt behind
        for kt in range(NTS):
            ps = psA.tile([P, max(S, main_w)], f32, tag="psA", name=f"sc_{h}_{kt}")
            lhsT = kT[ph : ph + D, eh * S + kt * P : eh * S + (kt + 1) * P]
            for j in range(NQ):
                nc.tensor.matmul(
                    ps[:, j * FD : (j + 1) * FD],
                    lhsT,
                    qT[ph : ph + D, eh * S + j * FD : eh * S + (j + 1) * FD],
                    start=True,
                    stop=True,
                )
            at = atp.tile([P, S], bf16, tag="at", name=f"at_{h}_{kt}")
            nc.scalar.activation(
                out=at,
                in_=ps[:, :S],
                func=mybir.ActivationFunctionType.Exp,
                scale=inv_scale,
            )
            ats.append(at)

            def o_mms(okt):
                lhsTo = vA[:, okt * HA + h * DA : okt * HA + (h + 1) * DA]
                for j in range(NQ):
                    nc.tensor.matmul(
                        pso[:DA, j * FD : (j + 1) * FD],
                        lhsTo,
                        ats[okt][:, j * FD : (j + 1) * FD],
                        start=(okt == 0),
                        stop=(okt == NTS - 1),
                    )

            if kt >= 2:
                o_mms(kt - 2)
        o_mms(NTS - 2)
        o_mms(NTS - 1)
        # evacuate unnormalized o (bf16) + denominator row
        nc.vector.tensor_copy(
            out=oT[ph : ph + D, eh * S : (eh + 1) * S], in_=pso[:D, :S]
        )
        den_tmp = mini.tile([1, S], f32, tag="den_tmp")
        nc.vector.tensor_copy(out=den_tmp, in_=pso[D:DA, :S])
        den_f32_b, _ = den_tiles(h // hb)
        nc.gpsimd.dma_start(
            out=den_f32_b[h % hb : h % hb + 1, :], in_=den_tmp
        )
        if h % hb == hb - 1:
            b = h // hb
            nc.vector.reciprocal_approx_fast(
                out=den_batches[b], in_=den_batches[b]
            )
            nc.vector.tensor_copy(out=den_bfs[b], in_=den_batches[b])
            # normalize with ONE BATCH of delay: batch b-1's reciprocal chain
            # (DVE+DMA) has had a full batch of PE work to complete, so the
            # rb matmuls never stall the in-order PE queue.
            if b > 0:
                for et in range(((b - 1) * hb) // HPT, (b * hb) // HPT):
                    normalize_et(et)
            if h == H - 1:
                for et in range((b * hb) // HPT, (h + 1) // HPT):
                    normalize_et(et)
        # between the two heads of a pair, project the e-tile TWO ahead
        if h % HPT == 0 and eh + 2 < NTE:
            project_qk_et(qT, wq_bf, xq_bf, bq_sb, eh + 2, "q")
            project_qk_et(kT, wk_bf, xk_bf, bk_sb, eh + 2, "k")

    # ---- output projection: out[s, e] = sum_f oT[f, s] woT[f, e] + bo ----
    # The first two s-tiles split their kt loops: early kt matmuls (whose
    # oT e-tiles normalized long ago) run while the LAST norm batch's
    # reciprocal chain completes, hiding it.
    KSPLIT = max(NTE - HPT, 0)
    ost = {}

    def outproj_mms(st_i, kts, pool=None):
        if st_i not in ost:
            pl, tg = (pool, "psO") if pool is psO else (psA, "psA")
            ost[st_i] = (
                outp.tile([P, E], f32, tag="out", name=f"osb_{st_i}"),
                (pl or psA).tile(
                    [P, max(S, main_w)], f32, tag=tg, name=f"po_{st_i}"
                ),
            )
        osb, ps = ost[st_i]
        for kt in kts:
            lhsT = oT[:, kt * S + st_i * P : kt * S + st_i * P + P]
            for j in range(NE):
                nc.tensor.matmul(
                    ps[:, j * FD : (j + 1) * FD],
                    lhsT,
                    wo_bf[:, kt * E + j * FD : kt * E + (j + 1) * FD],
                    start=(kt == 0),
                    stop=(kt == NTE - 1),
                )
        if kts[-1] == NTE - 1:
            nc.vector.tensor_add(out=osb, in0=ps[:, :E], in1=bo_sb)
            nc.sync.dma_start(
                out=io["out"][st_i * P : (st_i + 1) * P, :], in_=osb
            )

    if NTS >= 4 and KSPLIT > 0:
        # four s-tiles' early-kt matmuls buffer the last norm batch's chain
        outproj_mms(0, list(range(KSPLIT)))
        outproj_mms(1, list(range(KSPLIT)))
        outproj_mms(2, list(range(KSPLIT)), pool=psO)
        outproj_mms(3, list(range(KSPLIT)), pool=psO)
        for s in range(4):
            outproj_mms(s, list(range(KSPLIT, NTE)))
        rest = range(4, NTS)
    elif NTS >= 2 and KSPLIT > 0:
        outproj_mms(0, list(range(KSPLIT)))
        outproj_mms(1, list(range(KSPLIT)))
        outproj_mms(0, list(range(KSPLIT, NTE)))
        outproj_mms(1, list(range(KSPLIT, NTE)))
        rest = range(2, NTS)
    else:
        rest = range(NTS)
    for st_i in rest:
        outproj_mms(st_i, list(range(NTE)))


def build_nc(S=1024, E=1024, H=16):
    key = (S, E, H)
    if key in _NC_CACHE:
        return _NC_CACHE[key]
    import concourse.tile as tile
    from concourse import bacc, mybir

    D = E // H
    HA = H * (D + 1)
    HPT = P // D
    f32 = mybir.dt.float32
    bf16 = mybir.dt.bfloat16
    nc = bacc.Bacc("TRN2", target_bir_lowering=False, debug=False)
    io = {}
    for name, shape, dt in [
        ("xqT", [E, S], bf16),
        ("xkT", [E, S], bf16),
        ("xvT", [E, S], bf16),
        ("wqT", [E, E], bf16),
        ("wkT", [E, E], bf16),
        ("wvTa", [E, HA], bf16),
        ("woT", [E, E], bf16),
        ("bq", [E], f32),
        ("bk", [E], f32),
        ("bva", [HA], f32),
        ("bo", [E], f32),
        ("sel", [HPT, P], bf16),
    ]:
        io[name] = nc.dram_tensor(name, shape, dt, kind="ExternalInput").ap()
    io["out"] = nc.dram_tensor("out", [S, E], f32, kind="ExternalOutput").ap()

    with tile.TileContext(nc) as tc:
        with ExitStack() as ctx:
            _emit(ctx, tc, io, S, E, H)
    nc.compile()
    _NC_CACHE[key] = nc
    return nc


def make_in_maps(queries, keys, values, Wq, bq, Wk, bk, Wv, bv, Wo, bo, H=16):
    """Host-side layout prep: transposes, bf16 casts, v augmentation."""
    import ml_dtypes

    N, S, E = queries.shape
    D = E // H
    DA = D + 1
    HA = H * DA
    HPT = P // D
    f32 = np.float32
    bf16 = ml_dtypes.bfloat16

    wqT = np.ascontiguousarray(np.asarray(Wq, f32).T.astype(bf16))
    wkT = np.ascontiguousarray(np.asarray(Wk, f32).T.astype(bf16))
    woT = np.ascontiguousarray(np.asarray(Wo, f32).T.astype(bf16))
    wvT = np.asarray(Wv, f32).T.astype(bf16)  # [f, e]
    wvTa = np.zeros((E, HA), bf16)
    bva = np.zeros((HA,), f32)
    bv = np.asarray(bv, f32)
    for h in range(H):
        wvTa[:, h * DA : h * DA + D] = wvT[:, h * D : (h + 1) * D]
        bva[h * DA : h * DA + D] = bv[h * D : (h + 1) * D]
        bva[h * DA + D] = 1.0  # ones column -> softmax denominator
    sel = np.zeros((HPT, P), bf16)
    for i in range(HPT):
        sel[i, i * D : (i + 1) * D] = 1.0
    shared = {
        "wqT": wqT,
        "wkT": wkT,
        "wvTa": wvTa,
        "woT": woT,
        "bq": np.ascontiguousarray(np.asarray(bq, f32)),
        "bk": np.ascontiguousarray(np.asarray(bk, f32)),
        "bva": bva,
        "bo": np.ascontiguousarray(np.asarray(bo, f32)),
        "sel": sel,
    }
    q = np.asarray(queries, f32)
    k = np.asarray(keys, f32)
    v = np.asarray(values, f32)
    in_maps = []
    for b in range(N):
        m = dict(shared)
        m["xqT"] = np.ascontiguousarray(q[b].T.astype(bf16))
        m["xkT"] = np.ascontiguousarray(k[b].T.astype(bf16))
        m["xvT"] = np.ascontiguousarray(v[b].T.astype(bf16))
        in_maps.append(m)
    return in_maps


def run(queries, keys, values, Wq, bq, Wk, bk, Wv, bv, Wo, bo, **spmd_kwargs):
    from concourse.bass_utils import run_bass_kernel_spmd

    queries = np.asarray(queries, np.float32)
    N, S, E = queries.shape
    H = 16
    nc = build_nc(S=S, E=E, H=H)
    in_maps = make_in_maps(queries, keys, values, Wq, bq, Wk, bk, Wv, bv, Wo, bo, H=H)
    res = run_bass_kernel_spmd(nc, in_maps, core_ids=list(range(N)), **spmd_kwargs)
    out = np.stack([res.results[b]["out"] for b in range(N)])
    return out.astype(np.float32), res


def kernel(queries, keys, values, Wq, bq, Wk, bk, Wv, bv, Wo, bo):
    out, _ = run(queries, keys, values, Wq, bq, Wk, bk, Wv, bv, Wo, bo)
    return out
